# revision 1
# baseline (speedup 1.0000x reference)
"""Trainium2 Bass kernel for nn_ContrastLoss (supervised-contrastive loss).

Reference computation (B=1024, D=128, C=100, K=32768, N=B+K=33792):
    l   = concat(labels, queue_label.T)          # [N, C]
    w   = labels @ l.T                           # [B, N] shared-class counts
    sim = query @ concat(keys, queue.T).T / T    # [B, N]
    logits = sim - rowmax(sim)
    denom  = sum(exp(logits) * logits_mask, 1)   # logits_mask zeros keys-diag
    loss = -(T/BT) * sqrt(w/max(w)) * (logits - log(denom))

Key restructurings used here:
  * max(w) == max_i rowsum(labels_i) exactly (binary labels, diag included),
    computed on-device from labels.
  * The softmax stabilizer need not be the true rowmax: inputs are
    L2-normalized so raw = q.d in [-1, 1]; a constant stabilizer m=1.0 is
    numerically safe.  This kills the rowmax pass and one collective.
  * loss = -(T/BT)/sqrt(wmax) * sqrt(w) * (raw - c_i)
         = -s .* ln(e .* 1/denom'),  with
      e = exp((raw - m)/T)  (stored),  denom' = rowsum(masked e),
      s = sqrt(w * (T/BT)^2 / wmax).
    The per-row 1/denom' folds into the ACT Log's per-partition scale.

Sharding: tensor-parallel over the N (similarity column) dimension.
Core c owns keys-block columns [c*128, (c+1)*128) and queue columns
[c*4096, (c+1)*4096) -> 4224 columns x all 1024 rows.  The keys-block
interleaving puts each core's self-diagonal in its own row-block c,
handled data-driven via a per-core diag-extractor mask.  Row-wise
denominators are combined with a single tiny AllGather ([128,8] f32).
"""

import numpy as np
import ml_dtypes

import concourse.bass as bass
import concourse.mybir as mybir
import concourse.tile as tile
from concourse import bacc, bass_isa
from concourse.bass_utils import run_bass_kernel_spmd

F32 = mybir.dt.float32
BF16 = mybir.dt.bfloat16
ALU = mybir.AluOpType
ACTF = mybir.ActivationFunctionType

B, D, C, KQ = 1024, 128, 100, 32768
NCORES = 8
RB = 8                 # row-blocks of 128 rows
KEYS_PC = B // NCORES  # 128 keys-block columns per core
Q_PC = KQ // NCORES    # 4096 queue columns per core
N_PC = KEYS_PC + Q_PC  # 4224 columns per core
CHUNK = 1408           # 3 chunks of 1408 = 4224; 1408 = 512+512+384 matmuls
NCHUNK = N_PC // CHUNK
MM_SPLITS = [(0, 512), (512, 512), (1024, 384)]
STAB = 1.0             # softmax stabilizer m (raw sim values are in [-1, 1])


def _build_nc(Tf: float, BTf: float, reps: int = 1, bench: bool = False):
    nc = bacc.Bacc("TRN2", target_bir_lowering=False, debug=False,
                   num_devices=NCORES)

    qT_d = nc.dram_tensor("qT", [128, B], F32, kind="ExternalInput")
    rhs_sim_d = nc.dram_tensor("rhs_sim", [128, N_PC], F32, kind="ExternalInput")
    labT_d = nc.dram_tensor("labT", [C, B], BF16, kind="ExternalInput")
    rhs_w_d = nc.dram_tensor("rhs_w", [C, N_PC], BF16, kind="ExternalInput")
    dmask_d = nc.dram_tensor("dmask", [128, RB, 128], F32, kind="ExternalInput")
    labels_d = nc.dram_tensor("labels", [B, C], F32, kind="ExternalInput")
    if bench:
        # timing-only: big result stays in device DRAM (not fetched over the
        # tunnel); a tiny external output keeps the pipeline observable.
        out_d = nc.dram_tensor("out_scratch", [B, N_PC], F32)
        outm_d = nc.dram_tensor("outm", [128, RB], F32, kind="ExternalOutput")
    else:
        out_d = nc.dram_tensor("out", [B, N_PC], F32, kind="ExternalOutput")

    with tile.TileContext(nc) as tc:
        with (
            tc.tile_pool(name="const", bufs=1) as const,
            tc.tile_pool(name="big", bufs=1) as big,
            tc.tile_pool(name="work", bufs=2) as work,
            tc.tile_pool(name="sq", bufs=2) as sq,
            tc.tile_pool(name="outp", bufs=3) as outp,
            tc.tile_pool(name="psum", bufs=2, space="PSUM") as psum,
            tc.tile_pool(name="dram", bufs=1, space="DRAM") as dram,
        ):
            for _rep in range(reps):
                # ---- constant loads -------------------------------------------
                qT = const.tile([128, B], F32)
                nc.sync.dma_start(out=qT[:], in_=qT_d[:])
                rhs_sim = const.tile([128, N_PC], F32)
                nc.sync.dma_start(out=rhs_sim[:], in_=rhs_sim_d[:])
                labT = const.tile([C, B], BF16)
                nc.sync.dma_start(out=labT[:], in_=labT_d[:])
                rhs_w = const.tile([C, N_PC], BF16)
                nc.sync.dma_start(out=rhs_w[:], in_=rhs_w_d[:])
                dmask = const.tile([128, RB, 128], F32)
                nc.sync.dma_start(out=dmask[:], in_=dmask_d[:])

                # ---- wmax = max_i rowsum(labels_i), on device -----------------
                labs = work.tile([128, RB, C], F32, tag="labs")
                nc.sync.dma_start(out=labs[:],
                                  in_=labels_d.rearrange("(r p) c -> p r c", p=128))
                rs = const.tile([128, RB], F32)
                nc.vector.tensor_reduce(rs[:], labs[:], axis=mybir.AxisListType.X,
                                        op=ALU.add)
                rsm = const.tile([128, 1], F32)
                nc.vector.tensor_reduce(rsm[:], rs[:], axis=mybir.AxisListType.X,
                                        op=ALU.max)
                gmax = const.tile([128, 1], F32)
                nc.gpsimd.partition_all_reduce(gmax[:], rsm[:], 128,
                                               bass_isa.ReduceOp.max)
                winv = const.tile([128, 1], F32)
                nc.vector.reciprocal(winv[:], gmax[:])
                # s = sqrt(w * (T/BT)^2 / wmax): ACT Sqrt per-partition scale
                sq_scale = const.tile([128, 1], F32)
                nc.vector.tensor_scalar_mul(sq_scale[:], winv[:], (Tf / BTf) ** 2)

                ebias = const.tile([128, 1], F32)
                nc.vector.memset(ebias, -STAB / Tf)
                zbias = const.tile([128, 1], F32)
                nc.vector.memset(zbias, 0.0)

                # ---- phase 1: sim matmul -> e = exp((raw-m)/T), rowsums -------
                e = big.tile([128, RB, N_PC], F32)
                acc3 = const.tile([128, RB, NCHUNK], F32)
                for rb in range(RB):
                    lhsT = qT[:, rb * 128:(rb + 1) * 128]
                    for k in range(NCHUNK):
                        base = k * CHUNK
                        ps = psum.tile([128, CHUNK], F32, tag="ps")
                        for (o, n) in MM_SPLITS:
                            nc.tensor.matmul(ps[:, o:o + n], lhsT,
                                             rhs_sim[:, base + o:base + o + n],
                                             start=True, stop=True)
                        nc.scalar.activation(e[:, rb, base:base + CHUNK], ps[:],
                                             ACTF.Exp, bias=ebias[:], scale=1.0 / Tf,
                                             accum_out=acc3[:, rb, k:k + 1])

                # ---- self-diagonal removal from denominators ------------------
                # corrneg[p, rb] = -e[p, rb, p] * dmask[p, rb, p]; dmask is zero
                # except in row-block c, so only that block gets corrected.
                corrneg = const.tile([128, RB], F32)
                ttr_dump = const.tile([128, RB, 128], F32)
                for rb in range(RB):
                    nc.vector.tensor_mul(ttr_dump[:, rb, :], e[:, rb, 0:128],
                                         dmask[:, rb, :])
                nc.vector.tensor_reduce(corrneg[:], ttr_dump[:],
                                        axis=mybir.AxisListType.X, op=ALU.add)
                dn = const.tile([128, RB], F32)
                nc.vector.tensor_reduce(dn[:], acc3[:], axis=mybir.AxisListType.X,
                                        op=ALU.add)
                dn2 = const.tile([128, RB], F32)
                # denom = rowsum(e) - diag  (corrneg holds +diag; subtract it)
                nc.vector.tensor_sub(dn2[:], dn[:], corrneg[:])

                if bench:
                    nc.sync.dma_start(out=outm_d[:], in_=dn2[:])
                # ---- cross-core denominator exchange (tiny AllGather) ---------
                dn_dram = dram.tile([128, RB], F32)
                gdn_dram = dram.tile([NCORES, 128, RB], F32, addr_space="Shared")
                nc.gpsimd.dma_start(out=dn_dram[:], in_=dn2[:])
                nc.gpsimd.collective_compute(
                    "AllGather", ALU.bypass,
                    replica_groups=[list(range(NCORES))],
                    ins=[dn_dram.opt()], outs=[gdn_dram.opt()],
                )
                gdn = const.tile([128, RB, NCORES], F32)
                nc.sync.dma_start(out=gdn[:], in_=gdn_dram.rearrange("g p r -> p r g"))
                denom = const.tile([128, RB], F32)
                nc.vector.tensor_reduce(denom[:], gdn[:], axis=mybir.AxisListType.X,
                                        op=ALU.add)
                invd = const.tile([128, RB], F32)
                nc.vector.reciprocal(invd[:], denom[:])

                # ---- phase 2: t = ln(e * invd) in place; w matmul; combine ----
                # All Ln passes first: Exp and Ln share one ACT table set,
                # and grouping keeps Sqrt's set swap to a single load instead
                # of one per row-block.
                for rb in range(RB):
                    nc.scalar.activation(e[:, rb, :], e[:, rb, :], ACTF.Ln,
                                         bias=zbias[:], scale=invd[:, rb:rb + 1])
                for rb in range(RB):
                    lw = labT[:, rb * 128:(rb + 1) * 128]
                    for k in range(NCHUNK):
                        base = k * CHUNK
                        psw = psum.tile([128, CHUNK], F32, tag="ps")
                        for (o, n) in MM_SPLITS:
                            nc.tensor.matmul(psw[:, o:o + n], lw,
                                             rhs_w[:, base + o:base + o + n],
                                             start=True, stop=True)
                        s = sq.tile([128, CHUNK], F32, tag="s")
                        nc.scalar.activation(s[:], psw[:], ACTF.Sqrt,
                                             bias=zbias[:], scale=sq_scale[:])
                        o_t = outp.tile([128, CHUNK], F32, tag="o")
                        # out = (t * -1) * s
                        nc.vector.scalar_tensor_tensor(
                            o_t[:], e[:, rb, base:base + CHUNK], -1.0, s[:],
                            op0=ALU.mult, op1=ALU.mult,
                        )
                        nc.sync.dma_start(
                            out=out_d[rb * 128:(rb + 1) * 128, base:base + CHUNK],
                            in_=o_t[:])
    nc.compile()
    return nc


def _host_prep(query, keys, labels, queue, queue_label):
    bf16 = ml_dtypes.bfloat16
    query = np.asarray(query, np.float32)
    keys = np.asarray(keys, np.float32)
    labels = np.asarray(labels, np.float32)
    queue = np.asarray(queue, np.float32)
    queue_label = np.asarray(queue_label, np.float32)

    qT = np.ascontiguousarray(query.T)                  # [128, B]
    labT16 = labels.T.astype(bf16)                      # [C, B] exact (0/1)
    ql16 = queue_label.astype(bf16)                     # [C, KQ] exact (0/1)

    in_maps = []
    for c in range(NCORES):
        kslice = slice(c * KEYS_PC, (c + 1) * KEYS_PC)
        qslice = slice(c * Q_PC, (c + 1) * Q_PC)
        rhs_sim = np.concatenate(
            [np.ascontiguousarray(keys[kslice].T), queue[:, qslice]], axis=1)
        rhs_w = np.concatenate([labT16[:, kslice], ql16[:, qslice]], axis=1)
        dmask = np.zeros((128, RB, 128), np.float32)
        idx = np.arange(128)
        dmask[idx, c, idx] = 1.0
        in_maps.append({
            "qT": qT,
            "rhs_sim": np.ascontiguousarray(rhs_sim, dtype=np.float32),
            "labT": np.ascontiguousarray(labT16),
            "rhs_w": np.ascontiguousarray(rhs_w),
            "dmask": dmask,
            "labels": labels,
        })
    return in_maps


def _gather_output(results):
    out = np.empty((B, B + KQ), np.float32)
    for c in range(NCORES):
        r = results[c]["out"]
        out[:, c * KEYS_PC:(c + 1) * KEYS_PC] = r[:, :KEYS_PC]
        out[:, B + c * Q_PC:B + (c + 1) * Q_PC] = r[:, KEYS_PC:]
    return out


def kernel(query, keys, labels, queue, queue_label, K, T, BT, **_unused):
    Tf = float(np.asarray(T))
    BTf = float(np.asarray(BT))
    nc = _build_nc(Tf, BTf)
    in_maps = _host_prep(query, keys, labels, queue, queue_label)
    res = run_bass_kernel_spmd(nc, in_maps, list(range(NCORES)))
    return _gather_output(res.results)


# Re-usable entry for test.py: returns (output, BassKernelResults) so the
# harness there can pull exec_time_ns / profile out of a traced run.
def kernel_traced(query, keys, labels, queue, queue_label, K, T, BT,
                  trace=False, **run_kwargs):
    Tf = float(np.asarray(T))
    BTf = float(np.asarray(BT))
    nc = _build_nc(Tf, BTf)
    in_maps = _host_prep(query, keys, labels, queue, queue_label)
    res = run_bass_kernel_spmd(nc, in_maps, list(range(NCORES)),
                               trace=trace, **run_kwargs)
    return _gather_output(res.results), res



# revision 4
# speedup vs baseline: 1.6193x; 1.6193x over previous
"""Trainium2 Bass kernel for nn_ContrastLoss (supervised-contrastive loss).

Reference computation (B=1024, D=128, C=100, K=32768, N=B+K=33792):
    l   = concat(labels, queue_label.T)          # [N, C]
    w   = labels @ l.T                           # [B, N] shared-class counts
    sim = query @ concat(keys, queue.T).T / T    # [B, N]
    logits = sim - rowmax(sim)
    denom  = sum(exp(logits) * logits_mask, 1)   # logits_mask zeros keys-diag
    loss = -(T/BT) * sqrt(w/max(w)) * (logits - log(denom))

Restructurings:
  * max(w) == max_i rowsum(labels_i) exactly (binary labels, diag of the
    keys block included), computed on-device from the labels.T block of
    rhs_w via a ones-vector matmul + free-dim max + broadcast matmul.
  * Constant softmax stabilizer m=1.0 (inputs are L2-normalized so raw
    sim is in [-1,1]); the shift cancels in log_prob exactly.
  * loss = s * (ln(denom') + m/T - raw/T), with s = sqrt(w*(T/BT)^2/wmax)
    >= 0 folding the sign and the w>0 mask (w=0 -> s=0).

Sharding: pure data-parallel over the B (query row) dimension -- core c
owns rows [c*128, (c+1)*128) and ALL N=33792 similarity columns, so
there are NO collectives and no cross-core dependencies at all (the
baseline spent ~70us/core in a start barrier + a [128,8] AllGather, and
is exposed to multi-core launch stagger through them).

Per-core pipeline (two passes over the columns, 33 chunks of 1024):
  pass1: sim matmul (bf16) -> ACT Exp (accum_out row-sums); the keys
         chunk keeps its e values for the self-diagonal correction.
  mid:   denom = sum(acc) - diag(e);  lnb = ln(denom) + m/T;
         wmax/scale via two tiny matmuls (ones trick).
  pass2: sim matmul again (recompute beats storing e: kills the full-
         matrix Ln pass) + w matmul (fp8, exact for 0/1 labels);
         s = ACT Sqrt(w * scale);  t = lnb - raw/T (alternating between
         ACT Identity and DVE tensor_scalar to balance the engines);
         out = t*s on DVE; staged output DMA on two queues.
"""

import numpy as np
import ml_dtypes

import concourse.bass as bass
import concourse.mybir as mybir
import concourse.tile as tile
from concourse import bacc
from concourse.bass_utils import run_bass_kernel_spmd

F32 = mybir.dt.float32
BF16 = mybir.dt.bfloat16
F8 = mybir.dt.float8e4
ALU = mybir.AluOpType
ACTF = mybir.ActivationFunctionType
AXX = mybir.AxisListType.X

B, D, C, KQ = 1024, 128, 100, 32768
NCORES = 8
RPC = B // NCORES          # 128 query rows per core
N = B + KQ                 # 33792 similarity columns, all on every core
CH = 1024                  # column chunk (psum tile: 2 banks)
NCH = N // CH              # 33
GRP = 3                    # chunks per output-DMA group
NGRP = NCH // GRP          # 11
STAB = 1.0                 # constant softmax stabilizer


def _build_nc(Tf: float, BTf: float):
    nc = bacc.Bacc("TRN2", target_bir_lowering=False, debug=False,
                   num_devices=NCORES)

    qT_d = nc.dram_tensor("qT", [D, RPC], BF16, kind="ExternalInput")
    rhs_sim_d = nc.dram_tensor("rhs_sim", [D, N], BF16, kind="ExternalInput")
    labT_d = nc.dram_tensor("labT", [C, RPC], F8, kind="ExternalInput")
    rhs_w_d = nc.dram_tensor("rhs_w", [C, N], F8, kind="ExternalInput")
    dmask_d = nc.dram_tensor("dmask", [RPC, B], F32, kind="ExternalInput")
    out_d = nc.dram_tensor("out", [RPC, N], F32, kind="ExternalOutput")

    with tile.TileContext(nc) as tc:
        with (
            tc.tile_pool(name="const", bufs=1) as const,
            tc.tile_pool(name="esc", bufs=2) as esc,
            tc.tile_pool(name="spool", bufs=3) as spool,
            tc.tile_pool(name="tpool", bufs=3) as tpool,
            tc.tile_pool(name="stg", bufs=2) as stg,
            tc.tile_pool(name="psA", bufs=2, space="PSUM") as psA,
            tc.tile_pool(name="psB", bufs=2, space="PSUM") as psB,
        ):
            # ---- input DMAs: rhs_sim on the sync queue, rhs_w on gpsimd ---
            qTc = const.tile([D, RPC], BF16)
            nc.sync.dma_start(out=qTc[:], in_=qT_d[:])
            dmask = const.tile([RPC, B], F32)
            nc.sync.dma_start(out=dmask[:], in_=dmask_d[:])
            rhs_sim = const.tile([D, N], BF16)
            for a in range(0, N, 4096):
                b = min(a + 4096, N)
                nc.sync.dma_start(out=rhs_sim[:, a:b], in_=rhs_sim_d[:, a:b])

            labTc = const.tile([C, RPC], F8)
            nc.gpsimd.dma_start(out=labTc[:], in_=labT_d[:])
            rhs_w = const.tile([C, N], F8)
            nc.gpsimd.dma_start(out=rhs_w[:, 0:B], in_=rhs_w_d[:, 0:B])
            for a in range(B, N, 8192):
                nc.gpsimd.dma_start(out=rhs_w[:, a:a + 8192],
                                    in_=rhs_w_d[:, a:a + 8192])

            ebias = const.tile([RPC, 1], F32)
            nc.vector.memset(ebias, -STAB / Tf)

            # ---- pass 1: e = exp((raw-m)/T), row-sums via accum_out -------
            e_keys = const.tile([RPC, B], F32)
            acc = const.tile([RPC, NCH], F32)
            for k in range(NCH):
                base = k * CH
                ps = psA.tile([RPC, CH], F32, tag="ps")
                nc.tensor.matmul(ps[:, 0:512], qTc[:],
                                 rhs_sim[:, base:base + 512],
                                 start=True, stop=True)
                nc.tensor.matmul(ps[:, 512:CH], qTc[:],
                                 rhs_sim[:, base + 512:base + CH],
                                 start=True, stop=True)
                if k == 0:
                    eo = e_keys[:]
                else:
                    et = esc.tile([RPC, CH], F32, tag="e")
                    eo = et[:]
                nc.scalar.activation(eo, ps[:], ACTF.Exp,
                                     bias=ebias[:], scale=1.0 / Tf,
                                     accum_out=acc[:, k:k + 1])

            # ---- denominator: subtract self-diagonal, take ln -------------
            tmp = const.tile([RPC, B], F32)
            nc.vector.tensor_mul(tmp[:], e_keys[:], dmask[:])
            corr = const.tile([RPC, 1], F32)
            nc.vector.tensor_reduce(corr[:], tmp[:], axis=AXX, op=ALU.add)
            dn = const.tile([RPC, 1], F32)
            nc.vector.tensor_reduce(dn[:], acc[:], axis=AXX, op=ALU.add)
            dn2 = const.tile([RPC, 1], F32)
            nc.vector.tensor_sub(dn2[:], dn[:], corr[:])
            lnd = const.tile([RPC, 1], F32)
            nc.scalar.activation(lnd[:], dn2[:], ACTF.Ln)
            lnb = const.tile([RPC, 1], F32)
            nc.vector.tensor_scalar_add(lnb[:], lnd[:], STAB / Tf)

            # ---- wmax = max_i rowsum(labels_i), via two tiny matmuls ------
            ones_c = const.tile([C, 1], F8)
            nc.vector.memset(ones_c, 1.0)
            ones_r = const.tile([1, RPC], F32)
            nc.vector.memset(ones_r, 1.0)
            pm = psA.tile([RPC, CH], F32, tag="ps")
            nc.tensor.matmul(pm[0:1, 0:512], ones_c[:], rhs_w[:, 0:512],
                             start=True, stop=True)
            nc.tensor.matmul(pm[0:1, 512:B], ones_c[:], rhs_w[:, 512:B],
                             start=True, stop=True)
            wm = const.tile([1, 1], F32)
            nc.vector.tensor_reduce(wm[:], pm[0:1, 0:B], axis=AXX, op=ALU.max)
            pb = psA.tile([RPC, CH], F32, tag="ps")
            nc.tensor.matmul(pb[:, 0:1], ones_r[:], wm[:],
                             start=True, stop=True)
            winv = const.tile([RPC, 1], F32)
            nc.vector.reciprocal(winv[:], pb[:, 0:1])
            sq_scale = const.tile([RPC, 1], F32)
            nc.vector.tensor_scalar_mul(sq_scale[:], winv[:], (Tf / BTf) ** 2)

            # ---- pass 2: recompute raw, s = sqrt(w*c), out = (lnb-raw/T)*s
            for k in range(NCH):
                base = k * CH
                g, j = divmod(k, GRP)
                if j == 0:
                    st = stg.tile([RPC, GRP * CH], F32, tag="st")
                ps_s = psA.tile([RPC, CH], F32, tag="ps")
                nc.tensor.matmul(ps_s[:, 0:512], qTc[:],
                                 rhs_sim[:, base:base + 512],
                                 start=True, stop=True)
                nc.tensor.matmul(ps_s[:, 512:CH], qTc[:],
                                 rhs_sim[:, base + 512:base + CH],
                                 start=True, stop=True)
                ps_w = psB.tile([RPC, CH], F32, tag="pw")
                nc.tensor.matmul(ps_w[:, 0:512], labTc[:],
                                 rhs_w[:, base:base + 512],
                                 start=True, stop=True)
                nc.tensor.matmul(ps_w[:, 512:CH], labTc[:],
                                 rhs_w[:, base + 512:base + CH],
                                 start=True, stop=True)
                s = spool.tile([RPC, CH], F32, tag="s")
                nc.scalar.activation(s[:], ps_w[:], ACTF.Sqrt,
                                     scale=sq_scale[:])
                t = tpool.tile([RPC, CH], F32, tag="t")
                if k % 2 == 0:
                    nc.scalar.activation(t[:], ps_s[:], ACTF.Identity,
                                         bias=lnb[:], scale=-1.0 / Tf)
                else:
                    nc.vector.tensor_scalar(t[:], ps_s[:], -1.0 / Tf, lnb[:],
                                            ALU.mult, ALU.add)
                nc.vector.tensor_mul(st[:, j * CH:(j + 1) * CH], t[:], s[:])
                if j == GRP - 1:
                    eng = nc.sync if (g % 2 == 0) else nc.gpsimd
                    a = g * GRP * CH
                    eng.dma_start(out=out_d[:, a:a + GRP * CH], in_=st[:])
    nc.compile()
    return nc


def _host_prep(query, keys, labels, queue, queue_label):
    bf16 = ml_dtypes.bfloat16
    f8 = ml_dtypes.float8_e4m3fn
    query = np.asarray(query, np.float32)
    keys = np.asarray(keys, np.float32)
    labels = np.asarray(labels, np.float32)
    queue = np.asarray(queue, np.float32)
    queue_label = np.asarray(queue_label, np.float32)

    qT = np.ascontiguousarray(query.T.astype(bf16))           # [D, B]
    rhs_sim = np.concatenate([keys.T, queue], axis=1).astype(bf16)
    labT = np.ascontiguousarray(labels.T.astype(f8))          # [C, B] exact
    rhs_w = np.ascontiguousarray(
        np.concatenate([labels.T, queue_label], axis=1).astype(f8))

    in_maps = []
    idx = np.arange(RPC)
    for c in range(NCORES):
        rows = slice(c * RPC, (c + 1) * RPC)
        dmask = np.zeros((RPC, B), np.float32)
        dmask[idx, c * RPC + idx] = 1.0
        in_maps.append({
            "qT": np.ascontiguousarray(qT[:, rows]),
            "rhs_sim": rhs_sim,
            "labT": np.ascontiguousarray(labT[:, rows]),
            "rhs_w": rhs_w,
            "dmask": dmask,
        })
    return in_maps


def _gather_output(results):
    return np.concatenate([results[c]["out"] for c in range(NCORES)], axis=0)


def kernel(query, keys, labels, queue, queue_label, K, T, BT, **_unused):
    Tf = float(np.asarray(T))
    BTf = float(np.asarray(BT))
    nc = _build_nc(Tf, BTf)
    in_maps = _host_prep(query, keys, labels, queue, queue_label)
    res = run_bass_kernel_spmd(nc, in_maps, list(range(NCORES)))
    return _gather_output(res.results)


# Re-usable entry for test.py: returns (output, BassKernelResults) so the
# harness there can pull exec_time_ns / profile out of a traced run.
def kernel_traced(query, keys, labels, queue, queue_label, K, T, BT,
                  trace=False, **run_kwargs):
    Tf = float(np.asarray(T))
    BTf = float(np.asarray(BT))
    nc = _build_nc(Tf, BTf)
    in_maps = _host_prep(query, keys, labels, queue, queue_label)
    res = run_bass_kernel_spmd(nc, in_maps, list(range(NCORES)),
                               trace=trace, **run_kwargs)
    return _gather_output(res.results), res


# revision 10
# speedup vs baseline: 1.6325x; 1.0082x over previous
"""Trainium2 Bass kernel for nn_ContrastLoss (supervised-contrastive loss).

Reference computation (B=1024, D=128, C=100, K=32768, N=B+K=33792):
    l   = concat(labels, queue_label.T)          # [N, C]
    w   = labels @ l.T                           # [B, N] shared-class counts
    sim = query @ concat(keys, queue.T).T / T    # [B, N]
    logits = sim - rowmax(sim)
    denom  = sum(exp(logits) * logits_mask, 1)   # logits_mask zeros keys-diag
    loss = -(T/BT) * sqrt(w/max(w)) * (logits - log(denom))

Restructurings:
  * max(w) == max_i rowsum(labels_i) exactly (binary labels, diag of the
    keys block included), computed on-device from the labels.T block of
    rhs_w via a ones-vector matmul + free-dim max + broadcast matmul.
  * Constant softmax stabilizer m=1.0 (inputs are L2-normalized so raw
    sim is in [-1,1]); the shift cancels in log_prob exactly.
  * loss = s * (ln(denom') + m/T - raw/T), with s = sqrt(w*(T/BT)^2/wmax)
    >= 0 folding the sign and the w>0 mask (w=0 -> s=0).

Sharding: pure data-parallel over the B (query row) dimension -- core c
owns rows [c*128, (c+1)*128) and ALL N=33792 similarity columns, so
there are NO collectives and no cross-core dependencies at all (the
baseline spent ~70us/core in a start barrier + a [128,8] AllGather, and
is exposed to multi-core launch stagger through them).

Per-core pipeline (two passes over the columns, 33 chunks of 1024):
  pass1: sim matmul (bf16) -> ACT Exp (accum_out row-sums); the keys
         chunk keeps its e values for the self-diagonal correction.
  mid:   denom = sum(acc) - diag(e);  lnb = ln(denom) + m/T;
         wmax/scale via two tiny matmuls (ones trick).
  pass2: sim matmul again (recompute beats storing e: kills the full-
         matrix Ln pass) + w matmul (fp8, exact for 0/1 labels);
         s = ACT Sqrt(w * scale);  t = lnb - raw/T (alternating between
         ACT Identity and DVE tensor_scalar to balance the engines);
         out = t*s on DVE; staged output DMA on two queues.
"""

import numpy as np
import ml_dtypes

import concourse.bass as bass
import concourse.mybir as mybir
import concourse.tile as tile
from concourse import bacc
from concourse.bass_utils import run_bass_kernel_spmd

F32 = mybir.dt.float32
BF16 = mybir.dt.bfloat16
F8 = mybir.dt.float8e4
ALU = mybir.AluOpType
ACTF = mybir.ActivationFunctionType
AXX = mybir.AxisListType.X

B, D, C, KQ = 1024, 128, 100, 32768
NCORES = 8
RPC = B // NCORES          # 128 query rows per core
N = B + KQ                 # 33792 similarity columns, all on every core
CH = 1024                  # column chunk (psum tile: 2 banks)
NCH = N // CH              # 33
GRP = 3                    # chunks per output-DMA group
NGRP = NCH // GRP          # 11
STAB = 1.0                 # constant softmax stabilizer


def _build_nc(Tf: float, BTf: float):
    nc = bacc.Bacc("TRN2", target_bir_lowering=False, debug=False,
                   num_devices=NCORES)

    qT_d = nc.dram_tensor("qT", [D, RPC], BF16, kind="ExternalInput")
    rhs_sim_d = nc.dram_tensor("rhs_sim", [D, N], BF16, kind="ExternalInput")
    labT_d = nc.dram_tensor("labT", [C, RPC], F8, kind="ExternalInput")
    rhs_w_d = nc.dram_tensor("rhs_w", [C, N], F8, kind="ExternalInput")
    dmask_d = nc.dram_tensor("dmask", [RPC, B], F32, kind="ExternalInput")
    out_d = nc.dram_tensor("out", [RPC, N], F32, kind="ExternalOutput")

    with tile.TileContext(nc) as tc:
        with (
            tc.tile_pool(name="const", bufs=1) as const,
            tc.tile_pool(name="esc", bufs=2) as esc,
            tc.tile_pool(name="spool", bufs=3) as spool,
            tc.tile_pool(name="stg", bufs=2) as stg,
            tc.tile_pool(name="psA", bufs=2, space="PSUM") as psA,
            tc.tile_pool(name="psB", bufs=2, space="PSUM") as psB,
        ):
            # ---- input DMAs: rhs_sim on the sync queue, rhs_w on gpsimd ---
            qTc = const.tile([D, RPC], BF16)
            nc.sync.dma_start(out=qTc[:], in_=qT_d[:])
            dmask = const.tile([RPC, B], F32)
            nc.sync.dma_start(out=dmask[:], in_=dmask_d[:])
            rhs_sim = const.tile([D, N], BF16)
            for a in range(0, N, 4096):
                b = min(a + 4096, N)
                nc.sync.dma_start(out=rhs_sim[:, a:b], in_=rhs_sim_d[:, a:b])

            labTc = const.tile([C, RPC], F8)
            nc.gpsimd.dma_start(out=labTc[:], in_=labT_d[:])
            rhs_w = const.tile([C, N], F8)
            nc.gpsimd.dma_start(out=rhs_w[:, 0:B], in_=rhs_w_d[:, 0:B])
            for a in range(B, N, 8192):
                nc.gpsimd.dma_start(out=rhs_w[:, a:a + 8192],
                                    in_=rhs_w_d[:, a:a + 8192])

            ebias = const.tile([RPC, 1], F32)
            nc.vector.memset(ebias, -STAB / Tf)

            # ---- pass 1: e = exp((raw-m)/T), row-sums via accum_out -------
            e_keys = const.tile([RPC, B], F32)
            acc = const.tile([RPC, NCH], F32)
            for k in range(NCH):
                base = k * CH
                ps = psA.tile([RPC, CH], F32, tag="ps")
                nc.tensor.matmul(ps[:, 0:512], qTc[:],
                                 rhs_sim[:, base:base + 512],
                                 start=True, stop=True)
                nc.tensor.matmul(ps[:, 512:CH], qTc[:],
                                 rhs_sim[:, base + 512:base + CH],
                                 start=True, stop=True)
                if k == 0:
                    eo = e_keys[:]
                else:
                    et = esc.tile([RPC, CH], F32, tag="e")
                    eo = et[:]
                # qT is pre-scaled by -1/T on the host, so psum = -raw/T and
                # e = exp(-(psum) - STAB/T).
                nc.scalar.activation(eo, ps[:], ACTF.Exp,
                                     bias=ebias[:], scale=-1.0,
                                     accum_out=acc[:, k:k + 1])

            # ---- denominator: subtract self-diagonal, take ln -------------
            tmp = const.tile([RPC, B], F32)
            nc.vector.tensor_mul(tmp[:], e_keys[:], dmask[:])
            corr = const.tile([RPC, 1], F32)
            nc.vector.tensor_reduce(corr[:], tmp[:], axis=AXX, op=ALU.add)
            dn = const.tile([RPC, 1], F32)
            nc.vector.tensor_reduce(dn[:], acc[:], axis=AXX, op=ALU.add)
            dn2 = const.tile([RPC, 1], F32)
            nc.vector.tensor_sub(dn2[:], dn[:], corr[:])
            lnd = const.tile([RPC, 1], F32)
            nc.scalar.activation(lnd[:], dn2[:], ACTF.Ln)
            lnb = const.tile([RPC, 1], F32)
            nc.vector.tensor_scalar_add(lnb[:], lnd[:], STAB / Tf)

            # ---- wmax = max_i rowsum(labels_i), via two tiny matmuls ------
            ones_c = const.tile([C, 1], F8)
            nc.vector.memset(ones_c, 1.0)
            ones_r = const.tile([1, RPC], F32)
            nc.vector.memset(ones_r, 1.0)
            pm = psA.tile([RPC, CH], F32, tag="ps")
            nc.tensor.matmul(pm[0:1, 0:512], ones_c[:], rhs_w[:, 0:512],
                             start=True, stop=True)
            nc.tensor.matmul(pm[0:1, 512:B], ones_c[:], rhs_w[:, 512:B],
                             start=True, stop=True)
            wm = const.tile([1, 1], F32)
            nc.vector.tensor_reduce(wm[:], pm[0:1, 0:B], axis=AXX, op=ALU.max)
            pb = psA.tile([RPC, CH], F32, tag="ps")
            nc.tensor.matmul(pb[:, 0:1], ones_r[:], wm[:],
                             start=True, stop=True)
            winv = const.tile([RPC, 1], F32)
            nc.vector.reciprocal(winv[:], pb[:, 0:1])
            sq_scale = const.tile([RPC, 1], F32)
            nc.vector.tensor_scalar_mul(sq_scale[:], winv[:], (Tf / BTf) ** 2)

            # ---- pass 2: recompute raw, s = sqrt(w*c), out = (lnb-raw/T)*s
            for k in range(NCH):
                base = k * CH
                g, j = divmod(k, GRP)
                if j == 0:
                    st = stg.tile([RPC, GRP * CH], F32, tag="st")
                ps_s = psA.tile([RPC, CH], F32, tag="ps")
                nc.tensor.matmul(ps_s[:, 0:512], qTc[:],
                                 rhs_sim[:, base:base + 512],
                                 start=True, stop=True)
                nc.tensor.matmul(ps_s[:, 512:CH], qTc[:],
                                 rhs_sim[:, base + 512:base + CH],
                                 start=True, stop=True)
                ps_w = psB.tile([RPC, CH], F32, tag="pw")
                nc.tensor.matmul(ps_w[:, 0:512], labTc[:],
                                 rhs_w[:, base:base + 512],
                                 start=True, stop=True)
                nc.tensor.matmul(ps_w[:, 512:CH], labTc[:],
                                 rhs_w[:, base + 512:base + CH],
                                 start=True, stop=True)
                s = spool.tile([RPC, CH], F32, tag="s")
                nc.scalar.activation(s[:], ps_w[:], ACTF.Sqrt,
                                     scale=sq_scale[:])
                # psum = -raw/T, so out = (psum + lnb) * s in one DVE op.
                nc.vector.scalar_tensor_tensor(
                    st[:, j * CH:(j + 1) * CH], ps_s[:], lnb[:], s[:],
                    op0=ALU.add, op1=ALU.mult)
                if j == GRP - 1:
                    eng = nc.sync if (g % 2 == 0) else nc.gpsimd
                    a = g * GRP * CH
                    eng.dma_start(out=out_d[:, a:a + GRP * CH], in_=st[:])
    nc.compile()
    return nc


def _host_prep(query, keys, labels, queue, queue_label, Tf):
    bf16 = ml_dtypes.bfloat16
    f8 = ml_dtypes.float8_e4m3fn
    query = np.asarray(query, np.float32)
    keys = np.asarray(keys, np.float32)
    labels = np.asarray(labels, np.float32)
    queue = np.asarray(queue, np.float32)
    queue_label = np.asarray(queue_label, np.float32)

    # Pre-scaled by -1/T: the sim matmul then produces -raw/T directly,
    # letting pass 2 fuse (lnb - raw/T)*s into one scalar_tensor_tensor.
    qT = np.ascontiguousarray((query.T * (-1.0 / Tf)).astype(bf16))
    rhs_sim = np.concatenate([keys.T, queue], axis=1).astype(bf16)
    labT = np.ascontiguousarray(labels.T.astype(f8))          # [C, B] exact
    rhs_w = np.ascontiguousarray(
        np.concatenate([labels.T, queue_label], axis=1).astype(f8))

    in_maps = []
    idx = np.arange(RPC)
    for c in range(NCORES):
        rows = slice(c * RPC, (c + 1) * RPC)
        dmask = np.zeros((RPC, B), np.float32)
        dmask[idx, c * RPC + idx] = 1.0
        in_maps.append({
            "qT": np.ascontiguousarray(qT[:, rows]),
            "rhs_sim": rhs_sim,
            "labT": np.ascontiguousarray(labT[:, rows]),
            "rhs_w": rhs_w,
            "dmask": dmask,
        })
    return in_maps


def _gather_output(results):
    return np.concatenate([results[c]["out"] for c in range(NCORES)], axis=0)


def kernel(query, keys, labels, queue, queue_label, K, T, BT, **_unused):
    Tf = float(np.asarray(T))
    BTf = float(np.asarray(BT))
    nc = _build_nc(Tf, BTf)
    in_maps = _host_prep(query, keys, labels, queue, queue_label, Tf)
    res = run_bass_kernel_spmd(nc, in_maps, list(range(NCORES)))
    return _gather_output(res.results)


# Re-usable entry for test.py: returns (output, BassKernelResults) so the
# harness there can pull exec_time_ns / profile out of a traced run.
def kernel_traced(query, keys, labels, queue, queue_label, K, T, BT,
                  trace=False, **run_kwargs):
    Tf = float(np.asarray(T))
    BTf = float(np.asarray(BT))
    nc = _build_nc(Tf, BTf)
    in_maps = _host_prep(query, keys, labels, queue, queue_label, Tf)
    res = run_bass_kernel_spmd(nc, in_maps, list(range(NCORES)),
                               trace=trace, **run_kwargs)
    return _gather_output(res.results), res
